# revision 33
# baseline (speedup 1.0000x reference)
"""GAT (dense masked softmax attention) Bass kernel for 8 Trainium2 NeuronCores.

Row-parallel sharding: core c owns output rows [c*NB, (c+1)*NB). Each core
computes the full h = x @ W.T (replicated) and its row-block of the masked
attention softmax against all N nodes in transposed layout (j on partitions,
own-rows i on free dim).

The pointwise softmax numerator exp(leaky_relu(s)), s = f1_i + f2_j (+mask),
uses the identity (exp is monotone, and 1+x >= e^x picks the right branch
for s<0 with <1% error on the linearized negative branch):

    z = max( exp(s), 1 + 0.01*s )           s >= 0 -> exp(s) wins exactly
                                            s <  0 -> 1+0.01s ~ exp(0.01s)

with m4 = 0.01*(f1 + f2 + amask) packed ON HOST into the DMA'd mask tensor
(f16, additive amask = -30000 -> exp==0 and 1+0.01s<0 for masked entries).
On device this is ONE wide ACT Exp (scale=100, no per-chunk bias) + ONE DVE
scalar_tensor_tensor per chunk:

    e1 = Exp(100 * m4)                      ACT, 4 chunks per instruction
    z  = (m4 + 1.0) max e1                  DVE, feeds PE directly

so the scalar engine runs a single pass over the N x NB block (the baseline
ran two: Prelu + Exp) and the mask multiply / f1 broadcast matmuls vanish.

    accT[Hh][q] += h_half.T @ z             h is the STATIONARY operand
    dn[q]       += ones.T @ (z0+z1)         pair-summed denominators

then out^T = elu(accT * (1/dn broadcast)), logits^T = fc_w @ out^T + b - all
transposed, no PE transposes anywhere. The dn reciprocal happens AFTER the
[1,NB] -> [128,NB] broadcast so it runs 128-partition-parallel.
"""

import contextlib
import ctypes
import sys
import types

import numpy as np
import ml_dtypes

import concourse.bacc as bacc
import concourse.mybir as mybir
import concourse.tile as tile

P = 128
AMASK = -30000.0  # additive mask pre-scaled by 0.01 on host -> -300 in m4


def _install_ntff_hook():
    """Register the axon NTFF profile hook so run_bass_kernel_spmd(trace=True)
    can capture neuron-profile data (antenv.axon_hooks is absent here)."""
    if "antenv.axon_hooks" in sys.modules:
        return
    try:
        lib = ctypes.CDLL("/opt/axon/libaxon_pjrt.so")
        if not hasattr(lib, "axon_start_nrt_profile"):
            return
    except OSError:
        return
    lib.axon_start_nrt_profile.argtypes = [ctypes.POINTER(ctypes.c_int64), ctypes.c_size_t]
    lib.axon_start_nrt_profile.restype = ctypes.c_int64
    lib.axon_stop_nrt_profile.argtypes = [ctypes.c_char_p]
    lib.axon_stop_nrt_profile.restype = ctypes.c_int64

    @contextlib.contextmanager
    def _hook(output_dir, device_ids):
        import jax

        jax.devices()
        if device_ids:
            ids = (ctypes.c_int64 * len(device_ids))(*device_ids)
            rc = lib.axon_start_nrt_profile(ids, len(device_ids))
        else:
            rc = lib.axon_start_nrt_profile(None, 0)
        if rc != 0:
            raise RuntimeError(f"axon_start_nrt_profile rc={rc}")
        try:
            yield
        finally:
            n = lib.axon_stop_nrt_profile(str(output_dir).encode())
            print(f"ntff profile: {n} file(s) in {output_dir}", file=sys.stderr)

    mod = types.ModuleType("antenv.axon_hooks")
    mod.get_axon_ntff_profile_hook = lambda: _hook
    mod.set_axon_ntff_profile_hook = lambda h: None
    sys.modules["antenv.axon_hooks"] = mod


class GatConfig:
    def __init__(self, n=8192, d=512, h=256, c=16, n_cores=8,
                 ep=4, la=5, cast_act="dve", zs_pool=True, dn_delay=6):
        assert n % (n_cores * P) == 0 and d % P == 0 and h % P == 0
        self.n, self.d, self.h, self.c, self.n_cores = n, d, h, c, n_cores
        self.nb = n // n_cores          # own rows per core
        self.nch = n // P               # j-chunks (also m-tiles of h)
        self.ndc = d // P               # feature chunks
        self.ep = ep                    # chunks per wide Exp / m4 DMA
        self.la = la                    # software pipeline lookahead (chunks)
        self.cast_act = cast_act        # h psum->sbuf casts on ACT (else DVE)
        self.zs_pool = zs_pool          # alternate pair-sums onto GpSimd
        self.dn_delay = dn_delay        # chunks to delay dn matmuls (lets the
                                        # pair-sum engine finish before PE)
        self.n_warm = 32                # PE warm-up matmuls during DMA ramp

    def key(self):
        return (self.n, self.d, self.h, self.c, self.n_cores, self.ep,
                self.la, self.cast_act, self.zs_pool, self.dn_delay,
                self.n_warm)


def build_gat(cfg: GatConfig):
    """Build + compile the SPMD Bass program (identical on all cores)."""
    nc = bacc.Bacc("TRN2", target_bir_lowering=False, debug=False,
                   enable_asserts=False, num_devices=cfg.n_cores)
    N, D, H, C = cfg.n, cfg.d, cfg.h, cfg.c
    NB, NCH, NDC = cfg.nb, cfg.nch, cfg.ndc
    EP, LA, MB = cfg.ep, cfg.la, cfg.nb // P
    NH, NQ = H // P, NB // 512
    NG = NCH // EP                      # exp quads
    f32 = mybir.dt.float32
    bf16 = mybir.dt.bfloat16
    fp16 = mybir.dt.float16

    # m4 host layout: quad g is a contiguous [P, EP*NB] block (host shuffles
    # rows so partition p holds j = g*EP*P + s*P + p at free slot s).
    m4d = nc.dram_tensor("m4", [NG * P, EP * NB], fp16, kind="ExternalInput").ap()
    xT = nc.dram_tensor("xT", [D, N], bf16, kind="ExternalInput").ap()
    wT = nc.dram_tensor("wT", [D, H], bf16, kind="ExternalInput").ap()
    fcwT = nc.dram_tensor("fcwT", [H, C], bf16, kind="ExternalInput").ap()
    fcb = nc.dram_tensor("fcb", [C, 1], f32, kind="ExternalInput").ap()
    logitsT = nc.dram_tensor("logitsT", [C, NB], f32, kind="ExternalOutput").ap()

    AF = mybir.ActivationFunctionType
    OP = mybir.AluOpType

    with tile.TileContext(nc) as tc:
        with (
            tc.tile_pool(name="persist", bufs=1) as pp,
            tc.tile_pool(name="mwork", bufs=2) as mwp,
            tc.tile_pool(name="zwork", bufs=3) as zwp,
            tc.tile_pool(name="tail", bufs=2) as tp,
        ):
            # ---------------- resident inputs ----------------
            w_sb = []
            for dd in range(NDC):
                t = pp.tile([P, H], bf16, tag=f"w{dd}")
                nc.sync.dma_start(t[:], wT[dd * P:(dd + 1) * P, :])
                w_sb.append(t)
            fcw_sb = []
            for hh in range(NH):
                t = pp.tile([P, C], bf16, tag=f"fcw{hh}")
                nc.sync.dma_start(t[:], fcwT[hh * P:(hh + 1) * P, :])
                fcw_sb.append(t)
            fcb_sb = pp.tile([C, 1], f32, tag="fcb")
            nc.sync.dma_start(fcb_sb[:], fcb[:])

            h_sb = [pp.tile([P, H], bf16, tag=f"h{m}", name=f"h{m}")
                    for m in range(NCH)]
            onecol = pp.tile([P, 1], bf16, tag="onecol")
            nc.gpsimd.memset(onecol[:], 1.0)
            onerow = pp.tile([1, P], bf16, tag="onerow")
            nc.gpsimd.memset(onerow[:], 1.0)
            # dummy activation so the ~2.7us ACT table load overlaps the DMA
            # ramp instead of delaying the first real Exp
            warm = pp.tile([1, 1], f32, tag="warm")
            nc.scalar.activation(warm[:], w_sb[0][0:1, 0:1], AF.Exp)

            xtb = {}
            m4t = {}
            e1t = {}
            e2t = {}

            # accT[hh][q] [P, 512] (4 banks) + dn[q] rows (2 banks) +
            # h-pipeline psum (2 banks) = 8.  (PSUM matmul outputs are capped
            # at 512 f32 = one 2KB bank; 1024-wide outs are invalid ISA.)
            with tc.tile_pool(name="acc", bufs=1, space="PSUM") as accp:
                accT = [[accp.tile([P, 512], f32, tag=f"accT{hh}_{q}",
                                   name=f"accT{hh}_{q}")
                         for q in range(NQ)] for hh in range(NH)]
                dn = [accp.tile([1, 512], f32, tag=f"dn{q}", name=f"dn{q}")
                      for q in range(NQ)]

                with tc.tile_pool(name="ps1", bufs=2, space="PSUM") as ps1:
                    NBLK = NCH // MB
                    NG_ = NCH // EP

                    # keep the PE busy during the initial DMA ramp so the
                    # HAM clock-gate is fully open when real work arrives
                    for _ in range(cfg.n_warm):
                        nc.tensor.matmul(accT[0][0][:, 0:H],
                                         w_sb[0][:, 0:P], w_sb[1][:],
                                         start=True, stop=True)

                    def fetch_xtb(cb):
                        if cb >= NBLK or (0, cb) in xtb:
                            return
                        for dd in range(NDC):
                            t = mwp.tile([P, MB * P], bf16, tag=f"xtb{dd}",
                                         bufs=2, name=f"xtb{dd}_{cb}")
                            nc.sync.dma_start(
                                t[:], xT[dd * P:(dd + 1) * P,
                                         cb * MB * P:(cb + 1) * MB * P])
                            xtb[dd, cb] = t

                    def fetch_m4(g):
                        if g >= NG_ or g in m4t:
                            return
                        mt = mwp.tile([P, EP * NB], fp16, tag="m4",
                                      bufs=3, name=f"m4_{g}")
                        nc.sync.dma_start(mt[:], m4d[g * P:(g + 1) * P, :])
                        m4t[g] = mt
                        et = mwp.tile([P, EP * NB], bf16, tag="e1",
                                      bufs=3, name=f"e1_{g}")
                        nc.scalar.activation(et[:], mt[:], AF.Exp, scale=100.0)
                        e1t[g] = et
                        # linear branch e2 = 1 + m4 (single-scalar TS, 4x)
                        e2 = mwp.tile([P, EP * NB], bf16, tag="e2",
                                      bufs=3, name=f"e2_{g}")
                        nc.vector.tensor_scalar(out=e2[:], in0=mt[:],
                                                scalar1=1.0, scalar2=None,
                                                op0=OP.add)
                        e2t[g] = e2

                    def produce(ch):
                        cb, mi = divmod(ch, MB)
                        g, s = divmod(ch, EP)
                        if mi == 0:
                            fetch_xtb(cb)       # bootstrap (block 0 only)
                        if s == 0:
                            fetch_m4(g)
                        # prefetches go AFTER current-block fetches so the
                        # first compute of a block is never queued behind them
                        if mi == 1:
                            fetch_xtb(cb + 1)
                        if s == 1:
                            fetch_m4(g + 1)
                        # h chunk: hps = xtb_chunk.T @ W.T  (psum f32)
                        hps = ps1.tile([P, H], f32, tag="hps")
                        for dd in range(NDC):
                            nc.tensor.matmul(hps[:],
                                             xtb[dd, cb][:, mi * P:(mi + 1) * P],
                                             w_sb[dd][:],
                                             start=(dd == 0), stop=(dd == NDC - 1))
                        use_act = cfg.cast_act == "act" or \
                            (cfg.cast_act == "alt" and ch % 2 == 1)
                        if use_act:
                            nc.scalar.copy(h_sb[ch][:], hps[:])
                        else:
                            nc.vector.tensor_copy(h_sb[ch][:], hps[:])

                    NPAIR = NCH // 2
                    zpair = {}
                    zs_of = {}
                    dn_pending = []

                    def emit_dn(pr):
                        zs = zs_of.pop(pr)
                        for q in range(NQ):
                            nc.tensor.matmul(dn[q][:], onecol[:],
                                             zs[:, q * 512:q * 512 + 512],
                                             start=(pr == 0),
                                             stop=(pr == NPAIR - 1))

                    def consume(c):
                        g, s = divmod(c, EP)
                        pr, pe = divmod(c, 2)
                        while dn_pending and dn_pending[0][1] <= c - cfg.dn_delay:
                            emit_dn(dn_pending.pop(0)[0])
                        if pe == 0:
                            zpair[pr] = zwp.tile([P, 2 * NB], bf16, tag="z",
                                                 bufs=3, name=f"z{pr}")
                        zp = zpair[pr]
                        # z = e2 max e1   [one all-f16 DVE pass, 2x mode]
                        nc.vector.tensor_tensor(
                            out=zp[:, pe * NB:(pe + 1) * NB],
                            in0=e2t[g][:, s * NB:(s + 1) * NB],
                            in1=e1t[g][:, s * NB:(s + 1) * NB],
                            op=OP.max)
                        for hh in range(NH):
                            for q in range(NQ):
                                nc.tensor.matmul(
                                    accT[hh][q][:],
                                    h_sb[c][:, hh * P:(hh + 1) * P],
                                    zp[:, pe * NB + q * 512:pe * NB + q * 512 + 512],
                                    start=(c == 0), stop=(c == NCH - 1))
                        if pe == 1:
                            zs = zwp.tile([P, NB], bf16, tag="zs", bufs=3)
                            eng = nc.gpsimd if (cfg.zs_pool and pr % 2 == 0) \
                                else nc.vector
                            eng.tensor_tensor(out=zs[:], in0=zp[:, 0:NB],
                                              in1=zp[:, NB:2 * NB], op=OP.add)
                            zs_of[pr] = zs
                            dn_pending.append((pr, c))
                            zpair.pop(pr)

                    for ch in range(NCH):
                        produce(ch)
                        if ch >= LA:
                            consume(ch - LA)
                    for c in range(NCH - LA, NCH):
                        consume(c)
                    while dn_pending:
                        emit_dn(dn_pending.pop(0)[0])

                # ---- tail A: normalize + ELU (transposed layout) ----
                # broadcast dn to 128 partitions FIRST, then reciprocal
                # (128-way parallel instead of a 1-partition op)
                dnrow = pp.tile([1, NB], bf16, tag="dnrow")
                for q in range(NQ):
                    nc.vector.tensor_copy(dnrow[0:1, q * 512:q * 512 + 512],
                                          dn[q][:])
                rec = pp.tile([P, NB], f32, tag="rec")
                oeT = []
                with tc.tile_pool(name="psR", bufs=2, space="PSUM") as psR:
                    rs = tp.tile([P, NB], f32, tag="rs", bufs=1)
                    for q in range(NQ):
                        rb = psR.tile([P, 512], f32, tag="rb")
                        nc.tensor.matmul(rb[:], onerow[:],
                                         dnrow[0:1, q * 512:q * 512 + 512],
                                         start=True, stop=True)
                        nc.vector.tensor_copy(rs[:, q * 512:q * 512 + 512],
                                              rb[:])
                    nc.vector.reciprocal_approx_fast(rec[:], rs[:])
                    for hh in range(NH):
                        on = tp.tile([P, NB], f32, tag="on", bufs=2)
                        for q in range(NQ):
                            nc.vector.tensor_tensor(
                                out=on[:, q * 512:q * 512 + 512],
                                in0=accT[hh][q][:],
                                in1=rec[:, q * 512:q * 512 + 512],
                                op=OP.mult)
                        pos = tp.tile([P, NB], f32, tag="pos", bufs=2)
                        nc.vector.tensor_scalar(out=pos[:], in0=on[:],
                                                scalar1=0.0, scalar2=None,
                                                op0=OP.max)
                        ngm = tp.tile([P, NB], f32, tag="ngm", bufs=2)
                        nc.vector.tensor_scalar(out=ngm[:], in0=on[:],
                                                scalar1=0.0, scalar2=None,
                                                op0=OP.min)
                        ex = tp.tile([P, NB], f32, tag="ex", bufs=2)
                        nc.scalar.activation(ex[:], ngm[:], AF.Exp)
                        o = pp.tile([P, NB], bf16, tag=f"oeT{hh}",
                                    name=f"oeT{hh}")
                        nc.vector.scalar_tensor_tensor(out=o[:], in0=ex[:],
                                                       scalar=-1.0,
                                                       in1=pos[:],
                                                       op0=OP.add,
                                                       op1=OP.add)
                        oeT.append(o)

            # ---- tail B: logitsT = fc_w @ oeT + b (no transposes) ----
            logT = pp.tile([C, NB], f32, tag="logT")
            with tc.tile_pool(name="ps3", bufs=2, space="PSUM") as ps3:
                for q in range(NQ):
                    lps = ps3.tile([C, 512], f32, tag="lps")
                    for hh in range(NH):
                        nc.tensor.matmul(lps[:], fcw_sb[hh][:],
                                         oeT[hh][:, q * 512:q * 512 + 512],
                                         start=(hh == 0), stop=(hh == NH - 1))
                    nc.vector.tensor_scalar(out=logT[:, q * 512:q * 512 + 512],
                                            in0=lps[:], scalar1=fcb_sb[:],
                                            scalar2=None, op0=OP.add)
            nc.sync.dma_start(logitsT[:], logT[:])

    nc.compile()
    return nc


# ---------------------------------------------------------------------------
# Host-side prep + execution
# ---------------------------------------------------------------------------

_CACHE = {}


def _get_nc(cfg: GatConfig):
    k = cfg.key()
    if k not in _CACHE:
        _CACHE[k] = build_gat(cfg)
    return _CACHE[k]


def prep_inputs(cfg, x, edge_index, W, a1, a2, fc_w, fc_b):
    """Shard + pack host inputs -> list of per-core in_maps."""
    bf = ml_dtypes.bfloat16
    N, NB, EP = cfg.n, cfg.nb, cfg.ep
    NG = cfg.nch // EP
    x = np.asarray(x, np.float32)
    W = np.asarray(W, np.float32)
    xT = np.ascontiguousarray(x.T).astype(bf)                    # [D, N]
    wT = np.ascontiguousarray(W.T).astype(bf)                    # [D, H]
    f1 = (x @ (W.T @ np.asarray(a1, np.float32))).ravel()        # [N]
    f2 = (x @ (W.T @ np.asarray(a2, np.float32))).ravel()        # [N]
    fcwT = np.ascontiguousarray(np.asarray(fc_w, np.float32).T).astype(bf)
    fcb = np.asarray(fc_b, np.float32).reshape(-1, 1)            # [C, 1]

    src = np.asarray(edge_index[0])
    dst = np.asarray(edge_index[1])
    diag = np.arange(NB)
    in_maps = []
    for c in range(cfg.n_cores):
        lo = c * NB
        # m4[j, i] = 0.01*(f1_i + f2_j) - 300*(not edge)   [f16]
        base = 0.01 * (f1[lo:lo + NB][None, :] + f2[:, None])
        m4 = base + 0.01 * AMASK
        sel = (src >= lo) & (src < lo + NB)
        js, is_ = dst[sel], src[sel] - lo
        m4[js, is_] = base[js, is_]
        m4[lo + diag, diag] = base[lo + diag, diag]
        # quad-major layout: [NG, P, EP, NB] so each quad DMA is contiguous
        m4q = np.ascontiguousarray(
            m4.reshape(NG, EP, P, NB).transpose(0, 2, 1, 3)
            .reshape(NG * P, EP * NB)).astype(np.float16)
        in_maps.append({
            "m4": m4q,
            "xT": xT,
            "wT": wT,
            "fcwT": fcwT,
            "fcb": fcb,
        })
    return in_maps


def run(cfg, inputs, trace=False):
    """Compile (cached), run on the 8 cores, return (logits, BassKernelResults)."""
    _install_ntff_hook()
    from concourse.bass_utils import run_bass_kernel_spmd

    nc = _get_nc(cfg)
    in_maps = prep_inputs(cfg, **inputs)
    res = run_bass_kernel_spmd(nc, in_maps, core_ids=list(range(cfg.n_cores)),
                               trace=trace)
    logits = np.concatenate(
        [np.asarray(res.results[c]["logitsT"], np.float32).T
         for c in range(cfg.n_cores)], axis=0)
    return logits, res


def kernel(x, edge_index, W, a1, a2, fc_w, fc_b):
    cfg = GatConfig(n=x.shape[0], d=x.shape[1], h=W.shape[0], c=fc_w.shape[0])
    logits, _ = run(cfg, dict(x=x, edge_index=edge_index, W=W, a1=a1, a2=a2,
                              fc_w=fc_w, fc_b=fc_b))
    return logits


# revision 34
# speedup vs baseline: 1.1041x; 1.1041x over previous
"""GAT (dense masked softmax attention) Bass kernel for 8 Trainium2 NeuronCores.

Row-parallel sharding: core c owns output rows [c*NB, (c+1)*NB). Each core
computes the full h = x @ W.T (replicated) and its row-block of the masked
attention softmax against all N nodes in transposed layout (j on partitions,
own-rows i on free dim).

The pointwise softmax numerator exp(leaky_relu(s)), s = f1_i + f2_j (+mask),
uses the identity (exp is monotone, and 1+x >= e^x picks the right branch
for s<0 with <1% error on the linearized negative branch):

    z = max( exp(s), 1 + 0.01*s )           s >= 0 -> exp(s) wins exactly
                                            s <  0 -> 1+0.01s ~ exp(0.01s)

with m4 = 0.01*(f1 + f2 + amask) packed ON HOST into the DMA'd mask tensor
(f16, additive amask = -30000 -> exp==0 and 1+0.01s<0 for masked entries).
On device this is ONE wide ACT Exp (scale=100, no per-chunk bias) + ONE DVE
scalar_tensor_tensor per chunk:

    e1 = Exp(100 * m4)                      ACT, 4 chunks per instruction
    z  = (m4 + 1.0) max e1                  DVE, feeds PE directly

so the scalar engine runs a single pass over the N x NB block (the baseline
ran two: Prelu + Exp) and the mask multiply / f1 broadcast matmuls vanish.

    accT[Hh][q] += h_half.T @ z             h is the STATIONARY operand
    dn[q]       += ones.T @ (z0+z1)         pair-summed denominators

then out^T = elu(accT * (1/dn broadcast)), logits^T = fc_w @ out^T + b - all
transposed, no PE transposes anywhere. The dn reciprocal happens AFTER the
[1,NB] -> [128,NB] broadcast so it runs 128-partition-parallel.
"""

import contextlib
import ctypes
import sys
import types

import numpy as np
import ml_dtypes

import concourse.bacc as bacc
import concourse.mybir as mybir
import concourse.tile as tile

P = 128
AMASK = -30000.0  # additive mask pre-scaled by 0.01 on host -> -300 in m4


def _install_ntff_hook():
    """Register the axon NTFF profile hook so run_bass_kernel_spmd(trace=True)
    can capture neuron-profile data (antenv.axon_hooks is absent here)."""
    if "antenv.axon_hooks" in sys.modules:
        return
    try:
        lib = ctypes.CDLL("/opt/axon/libaxon_pjrt.so")
        if not hasattr(lib, "axon_start_nrt_profile"):
            return
    except OSError:
        return
    lib.axon_start_nrt_profile.argtypes = [ctypes.POINTER(ctypes.c_int64), ctypes.c_size_t]
    lib.axon_start_nrt_profile.restype = ctypes.c_int64
    lib.axon_stop_nrt_profile.argtypes = [ctypes.c_char_p]
    lib.axon_stop_nrt_profile.restype = ctypes.c_int64

    @contextlib.contextmanager
    def _hook(output_dir, device_ids):
        import jax

        jax.devices()
        if device_ids:
            ids = (ctypes.c_int64 * len(device_ids))(*device_ids)
            rc = lib.axon_start_nrt_profile(ids, len(device_ids))
        else:
            rc = lib.axon_start_nrt_profile(None, 0)
        if rc != 0:
            raise RuntimeError(f"axon_start_nrt_profile rc={rc}")
        try:
            yield
        finally:
            n = lib.axon_stop_nrt_profile(str(output_dir).encode())
            print(f"ntff profile: {n} file(s) in {output_dir}", file=sys.stderr)

    mod = types.ModuleType("antenv.axon_hooks")
    mod.get_axon_ntff_profile_hook = lambda: _hook
    mod.set_axon_ntff_profile_hook = lambda h: None
    sys.modules["antenv.axon_hooks"] = mod


class GatConfig:
    def __init__(self, n=8192, d=512, h=256, c=16, n_cores=8,
                 ep=4, la=7, cast_act="dve", zs_pool=False, dn_delay=2):
        assert n % (n_cores * P) == 0 and d % P == 0 and h % P == 0
        self.n, self.d, self.h, self.c, self.n_cores = n, d, h, c, n_cores
        self.nb = n // n_cores          # own rows per core
        self.nch = n // P               # j-chunks (also m-tiles of h)
        self.ndc = d // P               # feature chunks
        self.ep = ep                    # chunks per wide Exp / m4 DMA
        self.la = la                    # software pipeline lookahead (chunks)
        self.cast_act = cast_act        # h psum->sbuf casts on ACT (else DVE)
        self.zs_pool = zs_pool          # alternate pair-sums onto GpSimd
        self.dn_delay = dn_delay        # chunks to delay dn matmuls (lets the
                                        # pair-sum engine finish before PE)
        self.n_warm = 32                # PE warm-up matmuls during DMA ramp

    def key(self):
        return (self.n, self.d, self.h, self.c, self.n_cores, self.ep,
                self.la, self.cast_act, self.zs_pool, self.dn_delay,
                self.n_warm)


def build_gat(cfg: GatConfig):
    """Build + compile the SPMD Bass program (identical on all cores)."""
    nc = bacc.Bacc("TRN2", target_bir_lowering=False, debug=False,
                   enable_asserts=False, num_devices=cfg.n_cores)
    N, D, H, C = cfg.n, cfg.d, cfg.h, cfg.c
    NB, NCH, NDC = cfg.nb, cfg.nch, cfg.ndc
    EP, LA, MB = cfg.ep, cfg.la, cfg.nb // P
    NH, NQ = H // P, NB // 512
    NG = NCH // EP                      # exp quads
    f32 = mybir.dt.float32
    bf16 = mybir.dt.bfloat16
    fp16 = mybir.dt.float16

    # m4 host layout: quad g is a contiguous [P, EP*NB] block (host shuffles
    # rows so partition p holds j = g*EP*P + s*P + p at free slot s).
    m4d = nc.dram_tensor("m4", [NG * P, EP * NB], fp16, kind="ExternalInput").ap()
    xT = nc.dram_tensor("xT", [D, N], bf16, kind="ExternalInput").ap()
    wT = nc.dram_tensor("wT", [D, H], bf16, kind="ExternalInput").ap()
    fcwT = nc.dram_tensor("fcwT", [H, C], bf16, kind="ExternalInput").ap()
    fcb = nc.dram_tensor("fcb", [C, 1], f32, kind="ExternalInput").ap()
    logitsT = nc.dram_tensor("logitsT", [C, NB], f32, kind="ExternalOutput").ap()

    AF = mybir.ActivationFunctionType
    OP = mybir.AluOpType

    with tile.TileContext(nc) as tc:
        with (
            tc.tile_pool(name="persist", bufs=1) as pp,
            tc.tile_pool(name="mwork", bufs=2) as mwp,
            tc.tile_pool(name="zwork", bufs=3) as zwp,
            tc.tile_pool(name="tail", bufs=2) as tp,
        ):
            # ---------------- resident inputs ----------------
            w_sb = []
            for dd in range(NDC):
                t = pp.tile([P, H], bf16, tag=f"w{dd}")
                nc.sync.dma_start(t[:], wT[dd * P:(dd + 1) * P, :])
                w_sb.append(t)
            fcw_sb = []
            for hh in range(NH):
                t = pp.tile([P, C], bf16, tag=f"fcw{hh}")
                nc.sync.dma_start(t[:], fcwT[hh * P:(hh + 1) * P, :])
                fcw_sb.append(t)
            fcb_sb = pp.tile([C, 1], f32, tag="fcb")
            nc.sync.dma_start(fcb_sb[:], fcb[:])

            h_sb = [pp.tile([P, H], bf16, tag=f"h{m}", name=f"h{m}")
                    for m in range(NCH)]
            onecol = pp.tile([P, 1], bf16, tag="onecol")
            nc.gpsimd.memset(onecol[:], 1.0)
            onerow = pp.tile([1, P], bf16, tag="onerow")
            nc.gpsimd.memset(onerow[:], 1.0)
            # dummy activation so the ~2.7us ACT table load overlaps the DMA
            # ramp instead of delaying the first real Exp
            warm = pp.tile([1, 1], f32, tag="warm")
            nc.scalar.activation(warm[:], w_sb[0][0:1, 0:1], AF.Exp)

            xtb = {}
            m4t = {}
            e1t = {}
            e2t = {}

            # accT[hh][q] [P, 512] (4 banks) + dn[q] rows (2 banks) +
            # h-pipeline psum (2 banks) = 8.  (PSUM matmul outputs are capped
            # at 512 f32 = one 2KB bank; 1024-wide outs are invalid ISA.)
            with tc.tile_pool(name="acc", bufs=1, space="PSUM") as accp:
                accT = [[accp.tile([P, 512], f32, tag=f"accT{hh}_{q}",
                                   name=f"accT{hh}_{q}")
                         for q in range(NQ)] for hh in range(NH)]
                dn = [accp.tile([1, 512], f32, tag=f"dn{q}", name=f"dn{q}")
                      for q in range(NQ)]

                with tc.tile_pool(name="ps1", bufs=2, space="PSUM") as ps1:
                    NBLK = NCH // MB
                    NG_ = NCH // EP

                    # keep the PE busy during the initial DMA ramp so the
                    # HAM clock-gate is fully open when real work arrives
                    for _ in range(cfg.n_warm):
                        nc.tensor.matmul(accT[0][0][:, 0:H],
                                         w_sb[0][:, 0:P], w_sb[1][:],
                                         start=True, stop=True)

                    def fetch_xtb(cb):
                        if cb >= NBLK or (0, cb) in xtb:
                            return
                        for dd in range(NDC):
                            t = mwp.tile([P, MB * P], bf16, tag=f"xtb{dd}",
                                         bufs=2, name=f"xtb{dd}_{cb}")
                            nc.sync.dma_start(
                                t[:], xT[dd * P:(dd + 1) * P,
                                         cb * MB * P:(cb + 1) * MB * P])
                            xtb[dd, cb] = t

                    def fetch_m4(g):
                        if g >= NG_ or g in m4t:
                            return
                        mt = mwp.tile([P, EP * NB], fp16, tag="m4",
                                      bufs=3, name=f"m4_{g}")
                        nc.sync.dma_start(mt[:], m4d[g * P:(g + 1) * P, :])
                        m4t[g] = mt
                        et = mwp.tile([P, EP * NB], bf16, tag="e1",
                                      bufs=3, name=f"e1_{g}")
                        nc.scalar.activation(et[:], mt[:], AF.Exp, scale=100.0)
                        e1t[g] = et
                        # linear branch e2 = 1 + m4 (single-scalar TS, 4x)
                        e2 = mwp.tile([P, EP * NB], bf16, tag="e2",
                                      bufs=3, name=f"e2_{g}")
                        nc.vector.tensor_scalar(out=e2[:], in0=mt[:],
                                                scalar1=1.0, scalar2=None,
                                                op0=OP.add)
                        e2t[g] = e2

                    def produce(ch):
                        cb, mi = divmod(ch, MB)
                        g, s = divmod(ch, EP)
                        if s == 0:
                            fetch_m4(g)         # m4 first: the accT z-path is
                        if mi == 0:             # the long pole at kernel start
                            fetch_xtb(cb)
                        # prefetches go AFTER current-block fetches so the
                        # first compute of a block is never queued behind them
                        if mi == 1:
                            fetch_xtb(cb + 1)
                        if s == 1:
                            fetch_m4(g + 1)
                        # h chunk: hps = xtb_chunk.T @ W.T  (psum f32)
                        hps = ps1.tile([P, H], f32, tag="hps")
                        for dd in range(NDC):
                            nc.tensor.matmul(hps[:],
                                             xtb[dd, cb][:, mi * P:(mi + 1) * P],
                                             w_sb[dd][:],
                                             start=(dd == 0), stop=(dd == NDC - 1))
                        use_act = cfg.cast_act == "act" or \
                            (cfg.cast_act == "alt" and ch % 2 == 1)
                        if use_act:
                            nc.scalar.copy(h_sb[ch][:], hps[:])
                        else:
                            nc.vector.tensor_copy(h_sb[ch][:], hps[:])

                    NPAIR = NCH // 2
                    zpair = {}
                    zs_of = {}
                    dn_pending = []

                    def emit_dn(pr):
                        zs = zs_of.pop(pr)
                        for q in range(NQ):
                            nc.tensor.matmul(dn[q][:], onecol[:],
                                             zs[:, q * 512:q * 512 + 512],
                                             start=(pr == 0),
                                             stop=(pr == NPAIR - 1))

                    def consume(c):
                        g, s = divmod(c, EP)
                        pr, pe = divmod(c, 2)
                        while dn_pending and dn_pending[0][1] <= c - cfg.dn_delay:
                            emit_dn(dn_pending.pop(0)[0])
                        if pe == 0:
                            zpair[pr] = zwp.tile([P, 2 * NB], bf16, tag="z",
                                                 bufs=3, name=f"z{pr}")
                        zp = zpair[pr]
                        # z = e2 max e1   [one all-f16 DVE pass, 2x mode]
                        nc.vector.tensor_tensor(
                            out=zp[:, pe * NB:(pe + 1) * NB],
                            in0=e2t[g][:, s * NB:(s + 1) * NB],
                            in1=e1t[g][:, s * NB:(s + 1) * NB],
                            op=OP.max)
                        for hh in range(NH):
                            for q in range(NQ):
                                nc.tensor.matmul(
                                    accT[hh][q][:],
                                    h_sb[c][:, hh * P:(hh + 1) * P],
                                    zp[:, pe * NB + q * 512:pe * NB + q * 512 + 512],
                                    start=(c == 0), stop=(c == NCH - 1))
                        if pe == 1:
                            zs = zwp.tile([P, NB], bf16, tag="zs", bufs=3)
                            eng = nc.gpsimd if (cfg.zs_pool and pr % 2 == 0) \
                                else nc.vector
                            eng.tensor_tensor(out=zs[:], in0=zp[:, 0:NB],
                                              in1=zp[:, NB:2 * NB], op=OP.add)
                            zs_of[pr] = zs
                            dn_pending.append((pr, c))
                            zpair.pop(pr)

                    for ch in range(NCH):
                        produce(ch)
                        if ch >= LA:
                            consume(ch - LA)
                    for c in range(NCH - LA, NCH):
                        consume(c)
                    while dn_pending:
                        emit_dn(dn_pending.pop(0)[0])

                # ---- tail A: normalize + ELU (transposed layout) ----
                # broadcast dn to 128 partitions FIRST, then reciprocal
                # (128-way parallel instead of a 1-partition op)
                dnrow = pp.tile([1, NB], bf16, tag="dnrow")
                for q in range(NQ):
                    nc.vector.tensor_copy(dnrow[0:1, q * 512:q * 512 + 512],
                                          dn[q][:])
                rec = pp.tile([P, NB], f32, tag="rec")
                oeT = []
                with tc.tile_pool(name="psR", bufs=2, space="PSUM") as psR:
                    rs = tp.tile([P, NB], f32, tag="rs", bufs=1)
                    for q in range(NQ):
                        rb = psR.tile([P, 512], f32, tag="rb")
                        nc.tensor.matmul(rb[:], onerow[:],
                                         dnrow[0:1, q * 512:q * 512 + 512],
                                         start=True, stop=True)
                        nc.vector.tensor_copy(rs[:, q * 512:q * 512 + 512],
                                              rb[:])
                    nc.vector.reciprocal_approx_fast(rec[:], rs[:])
                    for hh in range(NH):
                        on = tp.tile([P, NB], f32, tag="on", bufs=2)
                        for q in range(NQ):
                            nc.vector.tensor_tensor(
                                out=on[:, q * 512:q * 512 + 512],
                                in0=accT[hh][q][:],
                                in1=rec[:, q * 512:q * 512 + 512],
                                op=OP.mult)
                        pos = tp.tile([P, NB], f32, tag="pos", bufs=2)
                        nc.vector.tensor_scalar(out=pos[:], in0=on[:],
                                                scalar1=0.0, scalar2=None,
                                                op0=OP.max)
                        ngm = tp.tile([P, NB], f32, tag="ngm", bufs=2)
                        nc.vector.tensor_scalar(out=ngm[:], in0=on[:],
                                                scalar1=0.0, scalar2=None,
                                                op0=OP.min)
                        ex = tp.tile([P, NB], f32, tag="ex", bufs=2)
                        nc.scalar.activation(ex[:], ngm[:], AF.Exp)
                        o = pp.tile([P, NB], bf16, tag=f"oeT{hh}",
                                    name=f"oeT{hh}")
                        nc.vector.scalar_tensor_tensor(out=o[:], in0=ex[:],
                                                       scalar=-1.0,
                                                       in1=pos[:],
                                                       op0=OP.add,
                                                       op1=OP.add)
                        oeT.append(o)

            # ---- tail B: logitsT = fc_w @ oeT + b (no transposes) ----
            logT = pp.tile([C, NB], f32, tag="logT")
            with tc.tile_pool(name="ps3", bufs=2, space="PSUM") as ps3:
                for q in range(NQ):
                    lps = ps3.tile([C, 512], f32, tag="lps")
                    for hh in range(NH):
                        nc.tensor.matmul(lps[:], fcw_sb[hh][:],
                                         oeT[hh][:, q * 512:q * 512 + 512],
                                         start=(hh == 0), stop=(hh == NH - 1))
                    nc.vector.tensor_scalar(out=logT[:, q * 512:q * 512 + 512],
                                            in0=lps[:], scalar1=fcb_sb[:],
                                            scalar2=None, op0=OP.add)
            nc.sync.dma_start(logitsT[:], logT[:])

    nc.compile()
    return nc


# ---------------------------------------------------------------------------
# Host-side prep + execution
# ---------------------------------------------------------------------------

_CACHE = {}


def _get_nc(cfg: GatConfig):
    k = cfg.key()
    if k not in _CACHE:
        _CACHE[k] = build_gat(cfg)
    return _CACHE[k]


def prep_inputs(cfg, x, edge_index, W, a1, a2, fc_w, fc_b):
    """Shard + pack host inputs -> list of per-core in_maps."""
    bf = ml_dtypes.bfloat16
    N, NB, EP = cfg.n, cfg.nb, cfg.ep
    NG = cfg.nch // EP
    x = np.asarray(x, np.float32)
    W = np.asarray(W, np.float32)
    xT = np.ascontiguousarray(x.T).astype(bf)                    # [D, N]
    wT = np.ascontiguousarray(W.T).astype(bf)                    # [D, H]
    f1 = (x @ (W.T @ np.asarray(a1, np.float32))).ravel()        # [N]
    f2 = (x @ (W.T @ np.asarray(a2, np.float32))).ravel()        # [N]
    fcwT = np.ascontiguousarray(np.asarray(fc_w, np.float32).T).astype(bf)
    fcb = np.asarray(fc_b, np.float32).reshape(-1, 1)            # [C, 1]

    src = np.asarray(edge_index[0])
    dst = np.asarray(edge_index[1])
    diag = np.arange(NB)
    in_maps = []
    for c in range(cfg.n_cores):
        lo = c * NB
        # m4[j, i] = 0.01*(f1_i + f2_j) - 300*(not edge)   [f16]
        base = 0.01 * (f1[lo:lo + NB][None, :] + f2[:, None])
        m4 = base + 0.01 * AMASK
        sel = (src >= lo) & (src < lo + NB)
        js, is_ = dst[sel], src[sel] - lo
        m4[js, is_] = base[js, is_]
        m4[lo + diag, diag] = base[lo + diag, diag]
        # quad-major layout: [NG, P, EP, NB] so each quad DMA is contiguous
        m4q = np.ascontiguousarray(
            m4.reshape(NG, EP, P, NB).transpose(0, 2, 1, 3)
            .reshape(NG * P, EP * NB)).astype(np.float16)
        in_maps.append({
            "m4": m4q,
            "xT": xT,
            "wT": wT,
            "fcwT": fcwT,
            "fcb": fcb,
        })
    return in_maps


def run(cfg, inputs, trace=False):
    """Compile (cached), run on the 8 cores, return (logits, BassKernelResults)."""
    _install_ntff_hook()
    from concourse.bass_utils import run_bass_kernel_spmd

    nc = _get_nc(cfg)
    in_maps = prep_inputs(cfg, **inputs)
    res = run_bass_kernel_spmd(nc, in_maps, core_ids=list(range(cfg.n_cores)),
                               trace=trace)
    logits = np.concatenate(
        [np.asarray(res.results[c]["logitsT"], np.float32).T
         for c in range(cfg.n_cores)], axis=0)
    return logits, res


def kernel(x, edge_index, W, a1, a2, fc_w, fc_b):
    cfg = GatConfig(n=x.shape[0], d=x.shape[1], h=W.shape[0], c=fc_w.shape[0])
    logits, _ = run(cfg, dict(x=x, edge_index=edge_index, W=W, a1=a1, a2=a2,
                              fc_w=fc_w, fc_b=fc_b))
    return logits


# revision 37
# speedup vs baseline: 1.1177x; 1.0123x over previous
"""GAT (dense masked softmax attention) Bass kernel for 8 Trainium2 NeuronCores.

Row-parallel sharding: core c owns output rows [c*NB, (c+1)*NB). Each core
computes the full h = x @ W.T (replicated) and its row-block of the masked
attention softmax against all N nodes in transposed layout (j on partitions,
own-rows i on free dim).

The pointwise softmax numerator exp(leaky_relu(s)), s = f1_i + f2_j (+mask),
uses the identity (exp is monotone, and 1+x >= e^x picks the right branch
for s<0 with <1% error on the linearized negative branch):

    z = max( exp(s), 1 + 0.01*s )           s >= 0 -> exp(s) wins exactly
                                            s <  0 -> 1+0.01s ~ exp(0.01s)

with m4 = 0.01*(f1 + f2 + amask) packed ON HOST into the DMA'd mask tensor
(f16, additive amask = -30000 -> exp==0 and 1+0.01s<0 for masked entries).
On device this is ONE wide ACT Exp (scale=100, no per-chunk bias) + ONE DVE
scalar_tensor_tensor per chunk:

    e1 = Exp(100 * m4)                      ACT, 4 chunks per instruction
    z  = (m4 + 1.0) max e1                  DVE, feeds PE directly

so the scalar engine runs a single pass over the N x NB block (the baseline
ran two: Prelu + Exp) and the mask multiply / f1 broadcast matmuls vanish.

    accT[Hh][q] += h_half.T @ z             h is the STATIONARY operand
    dn[q]       += ones.T @ (z0+z1)         pair-summed denominators

then out^T = elu(accT * (1/dn broadcast)), logits^T = fc_w @ out^T + b - all
transposed, no PE transposes anywhere. The dn reciprocal happens AFTER the
[1,NB] -> [128,NB] broadcast so it runs 128-partition-parallel.
"""

import contextlib
import ctypes
import sys
import types

import numpy as np
import ml_dtypes

import concourse.bacc as bacc
import concourse.mybir as mybir
import concourse.tile as tile

P = 128
AMASK = -30000.0  # additive mask pre-scaled by 0.01 on host -> -300 in m4


def _install_ntff_hook():
    """Register the axon NTFF profile hook so run_bass_kernel_spmd(trace=True)
    can capture neuron-profile data (antenv.axon_hooks is absent here)."""
    if "antenv.axon_hooks" in sys.modules:
        return
    try:
        lib = ctypes.CDLL("/opt/axon/libaxon_pjrt.so")
        if not hasattr(lib, "axon_start_nrt_profile"):
            return
    except OSError:
        return
    lib.axon_start_nrt_profile.argtypes = [ctypes.POINTER(ctypes.c_int64), ctypes.c_size_t]
    lib.axon_start_nrt_profile.restype = ctypes.c_int64
    lib.axon_stop_nrt_profile.argtypes = [ctypes.c_char_p]
    lib.axon_stop_nrt_profile.restype = ctypes.c_int64

    @contextlib.contextmanager
    def _hook(output_dir, device_ids):
        import jax

        jax.devices()
        if device_ids:
            ids = (ctypes.c_int64 * len(device_ids))(*device_ids)
            rc = lib.axon_start_nrt_profile(ids, len(device_ids))
        else:
            rc = lib.axon_start_nrt_profile(None, 0)
        if rc != 0:
            raise RuntimeError(f"axon_start_nrt_profile rc={rc}")
        try:
            yield
        finally:
            n = lib.axon_stop_nrt_profile(str(output_dir).encode())
            print(f"ntff profile: {n} file(s) in {output_dir}", file=sys.stderr)

    mod = types.ModuleType("antenv.axon_hooks")
    mod.get_axon_ntff_profile_hook = lambda: _hook
    mod.set_axon_ntff_profile_hook = lambda h: None
    sys.modules["antenv.axon_hooks"] = mod


class GatConfig:
    def __init__(self, n=8192, d=512, h=256, c=16, n_cores=8,
                 ep=4, la=7, cast_act="act", zs_pool=False, dn_delay=3,
                 dn_quad=True):
        assert n % (n_cores * P) == 0 and d % P == 0 and h % P == 0
        self.n, self.d, self.h, self.c, self.n_cores = n, d, h, c, n_cores
        self.nb = n // n_cores          # own rows per core
        self.nch = n // P               # j-chunks (also m-tiles of h)
        self.ndc = d // P               # feature chunks
        self.ep = ep                    # chunks per wide Exp / m4 DMA
        self.la = la                    # software pipeline lookahead (chunks)
        self.cast_act = cast_act        # h psum->sbuf casts on ACT (else DVE)
        self.zs_pool = zs_pool          # alternate pair-sums onto GpSimd
        self.dn_delay = dn_delay        # chunks to delay dn matmuls (lets the
                                        # pair-sum engine finish before PE)
        self.dn_quad = dn_quad          # two-level z reduction before dn
        self.n_warm = 32                # PE warm-up matmuls during DMA ramp

    def key(self):
        return (self.n, self.d, self.h, self.c, self.n_cores, self.ep,
                self.la, self.cast_act, self.zs_pool, self.dn_delay,
                self.dn_quad, self.n_warm)


def build_gat(cfg: GatConfig):
    """Build + compile the SPMD Bass program (identical on all cores)."""
    nc = bacc.Bacc("TRN2", target_bir_lowering=False, debug=False,
                   enable_asserts=False, num_devices=cfg.n_cores)
    N, D, H, C = cfg.n, cfg.d, cfg.h, cfg.c
    NB, NCH, NDC = cfg.nb, cfg.nch, cfg.ndc
    EP, LA, MB = cfg.ep, cfg.la, cfg.nb // P
    NH, NQ = H // P, NB // 512
    NG = NCH // EP                      # exp quads
    f32 = mybir.dt.float32
    bf16 = mybir.dt.bfloat16
    fp16 = mybir.dt.float16

    # m4 host layout: quad g is a contiguous [P, EP*NB] block (host shuffles
    # rows so partition p holds j = g*EP*P + s*P + p at free slot s).
    m4d = nc.dram_tensor("m4", [NG * P, EP * NB], fp16, kind="ExternalInput").ap()
    xT = nc.dram_tensor("xT", [D, N], bf16, kind="ExternalInput").ap()
    wT = nc.dram_tensor("wT", [D, H], bf16, kind="ExternalInput").ap()
    fcwT = nc.dram_tensor("fcwT", [H, C], bf16, kind="ExternalInput").ap()
    fcb = nc.dram_tensor("fcb", [C, 1], f32, kind="ExternalInput").ap()
    logitsT = nc.dram_tensor("logitsT", [C, NB], f32, kind="ExternalOutput").ap()

    AF = mybir.ActivationFunctionType
    OP = mybir.AluOpType

    with tile.TileContext(nc) as tc:
        with (
            tc.tile_pool(name="persist", bufs=1) as pp,
            tc.tile_pool(name="mwork", bufs=2) as mwp,
            tc.tile_pool(name="zwork", bufs=3) as zwp,
            tc.tile_pool(name="tail", bufs=2) as tp,
        ):
            # ---------------- resident inputs ----------------
            w_sb = []
            for dd in range(NDC):
                t = pp.tile([P, H], bf16, tag=f"w{dd}")
                nc.sync.dma_start(t[:], wT[dd * P:(dd + 1) * P, :])
                w_sb.append(t)
            fcw_sb = []
            for hh in range(NH):
                t = pp.tile([P, C], bf16, tag=f"fcw{hh}")
                nc.sync.dma_start(t[:], fcwT[hh * P:(hh + 1) * P, :])
                fcw_sb.append(t)
            fcb_sb = pp.tile([C, 1], f32, tag="fcb")
            nc.sync.dma_start(fcb_sb[:], fcb[:])

            h_sb = [pp.tile([P, H], bf16, tag=f"h{m}", name=f"h{m}")
                    for m in range(NCH)]
            onecol = pp.tile([P, 1], bf16, tag="onecol")
            nc.gpsimd.memset(onecol[:], 1.0)
            onerow = pp.tile([1, P], bf16, tag="onerow")
            nc.gpsimd.memset(onerow[:], 1.0)
            # dummy activation so the ~2.7us ACT table load overlaps the DMA
            # ramp instead of delaying the first real Exp
            warm = pp.tile([1, 1], f32, tag="warm")
            nc.scalar.activation(warm[:], w_sb[0][0:1, 0:1], AF.Exp)

            xtb = {}
            m4t = {}
            e1t = {}
            e2t = {}

            # accT[hh][q] [P, 512] (4 banks) + dn[q] rows (2 banks) +
            # h-pipeline psum (2 banks) = 8.  (PSUM matmul outputs are capped
            # at 512 f32 = one 2KB bank; 1024-wide outs are invalid ISA.)
            with tc.tile_pool(name="acc", bufs=1, space="PSUM") as accp:
                accT = [[accp.tile([P, 512], f32, tag=f"accT{hh}_{q}",
                                   name=f"accT{hh}_{q}")
                         for q in range(NQ)] for hh in range(NH)]
                dn = [accp.tile([1, 512], f32, tag=f"dn{q}", name=f"dn{q}")
                      for q in range(NQ)]

                with tc.tile_pool(name="ps1", bufs=2, space="PSUM") as ps1:
                    NBLK = NCH // MB
                    NG_ = NCH // EP

                    # keep the PE busy during the initial DMA ramp so the
                    # HAM clock-gate is fully open when real work arrives;
                    # memset-sourced 1-col matmuls have no DMA dependency
                    for _ in range(cfg.n_warm):
                        nc.tensor.matmul(accT[0][0][0:1, 0:1],
                                         onecol[:], onecol[:],
                                         start=True, stop=True)

                    def fetch_xtb(cb):
                        if cb >= NBLK or (0, cb) in xtb:
                            return
                        for dd in range(NDC):
                            t = mwp.tile([P, MB * P], bf16, tag=f"xtb{dd}",
                                         bufs=2, name=f"xtb{dd}_{cb}")
                            nc.sync.dma_start(
                                t[:], xT[dd * P:(dd + 1) * P,
                                         cb * MB * P:(cb + 1) * MB * P])
                            xtb[dd, cb] = t

                    def fetch_m4(g):
                        if g >= NG_ or g in m4t:
                            return
                        mt = mwp.tile([P, EP * NB], fp16, tag="m4",
                                      bufs=3, name=f"m4_{g}")
                        nc.sync.dma_start(mt[:], m4d[g * P:(g + 1) * P, :])
                        m4t[g] = mt
                        et = mwp.tile([P, EP * NB], bf16, tag="e1",
                                      bufs=3, name=f"e1_{g}")
                        nc.scalar.activation(et[:], mt[:], AF.Exp, scale=100.0)
                        e1t[g] = et
                        # linear branch e2 = 1 + m4 (single-scalar TS, 4x)
                        e2 = mwp.tile([P, EP * NB], bf16, tag="e2",
                                      bufs=3, name=f"e2_{g}")
                        nc.vector.tensor_scalar(out=e2[:], in0=mt[:],
                                                scalar1=1.0, scalar2=None,
                                                op0=OP.add)
                        e2t[g] = e2

                    def produce(ch):
                        cb, mi = divmod(ch, MB)
                        g, s = divmod(ch, EP)
                        if s == 0:
                            fetch_m4(g)         # m4 first: the accT z-path is
                        if mi == 0:             # the long pole at kernel start
                            fetch_xtb(cb)
                        # prefetches go AFTER current-block fetches so the
                        # first compute of a block is never queued behind them
                        if mi == 1:
                            fetch_xtb(cb + 1)
                        if s == 1:
                            fetch_m4(g + 1)
                        # h chunk: hps = xtb_chunk.T @ W.T  (psum f32)
                        hps = ps1.tile([P, H], f32, tag="hps")
                        for dd in range(NDC):
                            nc.tensor.matmul(hps[:],
                                             xtb[dd, cb][:, mi * P:(mi + 1) * P],
                                             w_sb[dd][:],
                                             start=(dd == 0), stop=(dd == NDC - 1))
                        use_act = cfg.cast_act == "act" or \
                            (cfg.cast_act == "alt" and ch % 2 == 1)
                        if use_act:
                            nc.scalar.copy(h_sb[ch][:], hps[:])
                        else:
                            nc.vector.tensor_copy(h_sb[ch][:], hps[:])

                    NGRP = NCH // 4 if cfg.dn_quad else NCH // 2
                    zpair = {}
                    zs_of = {}
                    dn_pending = []

                    def emit_dn(grp):
                        zs = zs_of.pop(grp)
                        for q in range(NQ):
                            nc.tensor.matmul(dn[q][:], onecol[:],
                                             zs[:, q * 512:q * 512 + 512],
                                             start=(grp == 0),
                                             stop=(grp == NGRP - 1))

                    def consume(c):
                        g, s = divmod(c, EP)
                        pr, pe = divmod(c, 2)
                        while dn_pending and dn_pending[0][1] <= c - cfg.dn_delay:
                            emit_dn(dn_pending.pop(0)[0])
                        if pe == 0:
                            zpair[pr] = zwp.tile([P, 2 * NB], bf16, tag="z",
                                                 bufs=3, name=f"z{pr}")
                        zp = zpair[pr]
                        # z = e2 max e1   [one all-bf16 DVE pass, 2x mode]
                        nc.vector.tensor_tensor(
                            out=zp[:, pe * NB:(pe + 1) * NB],
                            in0=e2t[g][:, s * NB:(s + 1) * NB],
                            in1=e1t[g][:, s * NB:(s + 1) * NB],
                            op=OP.max)
                        for hh in range(NH):
                            for q in range(NQ):
                                nc.tensor.matmul(
                                    accT[hh][q][:],
                                    h_sb[c][:, hh * P:(hh + 1) * P],
                                    zp[:, pe * NB + q * 512:pe * NB + q * 512 + 512],
                                    start=(c == 0), stop=(c == NCH - 1))
                        if pe == 1:
                            eng = nc.gpsimd if (cfg.zs_pool and pr % 2 == 0) \
                                else nc.vector
                            if not cfg.dn_quad:
                                zs = zwp.tile([P, NB], bf16, tag="zs", bufs=3)
                                eng.tensor_tensor(out=zs[:], in0=zp[:, 0:NB],
                                                  in1=zp[:, NB:2 * NB],
                                                  op=OP.add)
                                zs_of[pr] = zs
                                dn_pending.append((pr, c))
                            else:
                                # two-level reduction: pair-sums, then a quad
                                # sum; dn matmuls stream 4 chunks' worth once
                                qd, qe = divmod(pr, 2)
                                zs = zwp.tile([P, NB], bf16, tag="zs", bufs=3,
                                              name=f"zs{pr}")
                                eng.tensor_tensor(out=zs[:], in0=zp[:, 0:NB],
                                                  in1=zp[:, NB:2 * NB],
                                                  op=OP.add)
                                zs_of[("p", pr)] = zs
                                if qe == 1:
                                    zq = zwp.tile([P, NB], bf16, tag="zq",
                                                  bufs=2, name=f"zq{qd}")
                                    nc.vector.tensor_tensor(
                                        out=zq[:],
                                        in0=zs_of.pop(("p", 2 * qd))[:],
                                        in1=zs_of.pop(("p", 2 * qd + 1))[:],
                                        op=OP.add)
                                    zs_of[qd] = zq
                                    dn_pending.append((qd, c))
                            zpair.pop(pr)

                    for ch in range(NCH):
                        produce(ch)
                        if ch >= LA:
                            consume(ch - LA)
                    for c in range(NCH - LA, NCH):
                        consume(c)
                    while dn_pending:
                        emit_dn(dn_pending.pop(0)[0])

                # ---- tail A: normalize + ELU (transposed layout) ----
                # broadcast dn to 128 partitions FIRST, then reciprocal
                # (128-way parallel instead of a 1-partition op)
                dnrow = pp.tile([1, NB], bf16, tag="dnrow")
                for q in range(NQ):
                    nc.vector.tensor_copy(dnrow[0:1, q * 512:q * 512 + 512],
                                          dn[q][:])
                rec = pp.tile([P, NB], f32, tag="rec")
                oeT = []
                with tc.tile_pool(name="psR", bufs=2, space="PSUM") as psR:
                    rs = tp.tile([P, NB], f32, tag="rs", bufs=1)
                    for q in range(NQ):
                        rb = psR.tile([P, 512], f32, tag="rb")
                        nc.tensor.matmul(rb[:], onerow[:],
                                         dnrow[0:1, q * 512:q * 512 + 512],
                                         start=True, stop=True)
                        nc.vector.tensor_copy(rs[:, q * 512:q * 512 + 512],
                                              rb[:])
                    nc.vector.reciprocal_approx_fast(rec[:], rs[:])
                    for hh in range(NH):
                        on = tp.tile([P, NB], f32, tag="on", bufs=2)
                        for q in range(NQ):
                            nc.vector.tensor_tensor(
                                out=on[:, q * 512:q * 512 + 512],
                                in0=accT[hh][q][:],
                                in1=rec[:, q * 512:q * 512 + 512],
                                op=OP.mult)
                        pos = tp.tile([P, NB], f32, tag="pos", bufs=2)
                        nc.vector.tensor_scalar(out=pos[:], in0=on[:],
                                                scalar1=0.0, scalar2=None,
                                                op0=OP.max)
                        ngm = tp.tile([P, NB], f32, tag="ngm", bufs=2)
                        nc.vector.tensor_scalar(out=ngm[:], in0=on[:],
                                                scalar1=0.0, scalar2=None,
                                                op0=OP.min)
                        ex = tp.tile([P, NB], f32, tag="ex", bufs=2)
                        nc.scalar.activation(ex[:], ngm[:], AF.Exp)
                        o = pp.tile([P, NB], bf16, tag=f"oeT{hh}",
                                    name=f"oeT{hh}")
                        nc.vector.scalar_tensor_tensor(out=o[:], in0=ex[:],
                                                       scalar=-1.0,
                                                       in1=pos[:],
                                                       op0=OP.add,
                                                       op1=OP.add)
                        oeT.append(o)

            # ---- tail B: logitsT = fc_w @ oeT + b (no transposes) ----
            logT = pp.tile([C, NB], f32, tag="logT")
            with tc.tile_pool(name="ps3", bufs=2, space="PSUM") as ps3:
                for q in range(NQ):
                    lps = ps3.tile([C, 512], f32, tag="lps")
                    for hh in range(NH):
                        nc.tensor.matmul(lps[:], fcw_sb[hh][:],
                                         oeT[hh][:, q * 512:q * 512 + 512],
                                         start=(hh == 0), stop=(hh == NH - 1))
                    nc.vector.tensor_scalar(out=logT[:, q * 512:q * 512 + 512],
                                            in0=lps[:], scalar1=fcb_sb[:],
                                            scalar2=None, op0=OP.add)
            nc.sync.dma_start(logitsT[:], logT[:])

    nc.compile()
    return nc


# ---------------------------------------------------------------------------
# Host-side prep + execution
# ---------------------------------------------------------------------------

_CACHE = {}


def _get_nc(cfg: GatConfig):
    k = cfg.key()
    if k not in _CACHE:
        _CACHE[k] = build_gat(cfg)
    return _CACHE[k]


def prep_inputs(cfg, x, edge_index, W, a1, a2, fc_w, fc_b):
    """Shard + pack host inputs -> list of per-core in_maps."""
    bf = ml_dtypes.bfloat16
    N, NB, EP = cfg.n, cfg.nb, cfg.ep
    NG = cfg.nch // EP
    x = np.asarray(x, np.float32)
    W = np.asarray(W, np.float32)
    xT = np.ascontiguousarray(x.T).astype(bf)                    # [D, N]
    wT = np.ascontiguousarray(W.T).astype(bf)                    # [D, H]
    f1 = (x @ (W.T @ np.asarray(a1, np.float32))).ravel()        # [N]
    f2 = (x @ (W.T @ np.asarray(a2, np.float32))).ravel()        # [N]
    fcwT = np.ascontiguousarray(np.asarray(fc_w, np.float32).T).astype(bf)
    fcb = np.asarray(fc_b, np.float32).reshape(-1, 1)            # [C, 1]

    src = np.asarray(edge_index[0])
    dst = np.asarray(edge_index[1])
    diag = np.arange(NB)
    in_maps = []
    for c in range(cfg.n_cores):
        lo = c * NB
        # m4[j, i] = 0.01*(f1_i + f2_j) - 300*(not edge)   [f16]
        base = 0.01 * (f1[lo:lo + NB][None, :] + f2[:, None])
        m4 = base + 0.01 * AMASK
        sel = (src >= lo) & (src < lo + NB)
        js, is_ = dst[sel], src[sel] - lo
        m4[js, is_] = base[js, is_]
        m4[lo + diag, diag] = base[lo + diag, diag]
        # quad-major layout: [NG, P, EP, NB] so each quad DMA is contiguous
        m4q = np.ascontiguousarray(
            m4.reshape(NG, EP, P, NB).transpose(0, 2, 1, 3)
            .reshape(NG * P, EP * NB)).astype(np.float16)
        in_maps.append({
            "m4": m4q,
            "xT": xT,
            "wT": wT,
            "fcwT": fcwT,
            "fcb": fcb,
        })
    return in_maps


def run(cfg, inputs, trace=False):
    """Compile (cached), run on the 8 cores, return (logits, BassKernelResults)."""
    _install_ntff_hook()
    from concourse.bass_utils import run_bass_kernel_spmd

    nc = _get_nc(cfg)
    in_maps = prep_inputs(cfg, **inputs)
    res = run_bass_kernel_spmd(nc, in_maps, core_ids=list(range(cfg.n_cores)),
                               trace=trace)
    logits = np.concatenate(
        [np.asarray(res.results[c]["logitsT"], np.float32).T
         for c in range(cfg.n_cores)], axis=0)
    return logits, res


def kernel(x, edge_index, W, a1, a2, fc_w, fc_b):
    cfg = GatConfig(n=x.shape[0], d=x.shape[1], h=W.shape[0], c=fc_w.shape[0])
    logits, _ = run(cfg, dict(x=x, edge_index=edge_index, W=W, a1=a1, a2=a2,
                              fc_w=fc_w, fc_b=fc_b))
    return logits
